# revision 36
# baseline (speedup 1.0000x reference)
"""Trainium2 Bass kernel for nn_Attention_13700945674736 (sparse local-window attention).

Strategy (8 NeuronCores, data-parallel over batch, 4 samples/core):
  - Permute the sequence axis s = 64*i + j  ->  s' = 16*j + i (image transpose).
    The 7x11 local window becomes a 1-D band |ds'| <= 83; key-chunk c of 128
    keys attends to queries in [128c-128, 128c+208) (low side extended from
    -80 to -128 so every @V piece is partition-base-0 legal); band width 2480.
  - QK^T band tiles [key, query] per head -> exp on ScalarE (3 chunk-groups)
    -> binary window mask multiplied on DVE (1 instr/head, bf16 2x mode).
  - @V is reoriented: for each query tile t (128 queries), matmul
    lhsT=expT piece [128 keys, <=128 q], rhs=vv[keys, 48 v | 1 ones] accumulates
    po2[t] = [128 q, 8 heads x 49] with the softmax denominator landing in
    column 48 of each head section.  Cost is 49 columns per piece instead of
    the full band -> @V PE time halves vs the [d, q] orientation.
  - Softmax normalize in [q, hd] layout: one reciprocal [128, 8] + one
    broadcast multiply [128, 8, 48] per q-tile (vs per-head [48, 1024]
    divides) -> ~3x less DVE work.
  - ao [q, (h d)] bf16 is transposed to aoT [dim, q] by a single
    dma_start_transpose per sample (xbar blocked transpose, runs on the idle
    DMA engines), giving DENSE 128-row dim blocks -> the output projection
    contracts in 3 matmuls instead of 4 (pair-padding eliminated).
  - b_proj is added on the host after the gather (free); the device computes
    out = ao_norm @ w_proj only.
  - Q/K heads stay padded to 64-partition slots (matmul operand partition
    bases must be 32-aligned); all matmul operands bf16, PSUM/softmax f32.
  - Software pipelining: attention(b) drains "filler" issue-units after each
    head and between @V tiles -- sample b's own deferred Q/K pair-2/3
    projection, proj(b+1)'s units, and out_proj(b-1)'s units -- so the PE
    always has independent matmul work while exp/mask/divide chains drain.
"""

import sys

sys.path.insert(0, "/opt/trn_rl_repo")

import numpy as np

import concourse.bass as bass
from concourse import bacc
import concourse.mybir as mybir
import concourse.tile as tile
from concourse.bass_utils import run_bass_kernel_spmd

# ---------------------------------------------------------------- constants
B, S, C = 32, 1024, 384
H, D = 8, 48
HI, WI = 16, 64
N_CORES = 8
BL = B // N_CORES  # samples per core
SCALE = float(D) ** -0.5
F32 = mybir.dt.float32
BF16 = mybir.dt.bfloat16

# s' = 16*j + i  <->  s = 64*i + j ;  PERM[s'] = s
_sp = np.arange(S)
PERM = (_sp % HI) * WI + (_sp // HI)

NQT = S // 128  # 8 query tiles (and key chunks)
WPADQ = 64 * H  # padded Q (and K) section width: 512
WQW = 2 * WPADQ + 3 * C  # padded QK (1024) + dense qkv (1152) = 2176

# per-chunk bands: key-chunk c attends to queries [QLO[c], QHI[c]).
# exact low edge is 128c-80; extended to 128c-128 so the piece of chunk c
# feeding query-tile c-1 starts at partition 0 (the extra columns are
# mask-zeroed so they contribute nothing).
QLO = [max(0, 128 * c - 128) for c in range(NQT)]
QHI = [min(S, 128 * c + 208) for c in range(NQT)]
WC = [QHI[c] - QLO[c] for c in range(NQT)]
OFFC = list(np.cumsum([0] + WC[:-1]))
BAND_W = sum(WC)  # 2480

# the band is cut into 1024-column slices; each is one 2-bank PSUM pat tile
N_SLICE = (BAND_W + 1023) // 1024  # 3
SLICE_W = [min(1024, BAND_W - 1024 * s) for s in range(N_SLICE)]
# chunks intersecting each slice: (chunk, band_lo, band_hi) in band coords
SLICE_CHUNKS = []
for _s in range(N_SLICE):
    slo, shi = 1024 * _s, 1024 * _s + SLICE_W[_s]
    cs = []
    for _c in range(NQT):
        lo = max(OFFC[_c], slo)
        hi = min(OFFC[_c] + WC[_c], shi)
        if lo < hi:
            cs.append((_c, lo, hi))
    SLICE_CHUNKS.append(cs)

# @V pieces per query tile t: (chunk c, expT col offset within chunk band,
# width).  Order: c=t first (full 128, start=True covers all partitions),
# then c=t+1 (full 128), then c=t-1 (80 wide); all partition-base 0.
AV_PIECES = []
for _t in range(NQT):
    ps = [(_t, 128 * _t - QLO[_t], 128)]
    if _t + 1 < NQT:
        ps.append((_t + 1, 128 * _t - QLO[_t + 1], 128))
    if _t > 0:
        ps.append((_t - 1, 128 * _t - QLO[_t - 1], QHI[_t - 1] - 128 * _t))
    AV_PIECES.append(ps)

# ------------------------------------------------- engine schedule tables
# 'a' = ScalarE(Act) copy, 'd' = DVE tensor_copy (Pool cannot touch PSUM)
QK_EVAC = "adadadadadadadad"  # 16 per sample (Q then K, pair-major)
V_EVAC = "dadddddd"  # 8 per sample
OUT_EVAC = "dadddddd"  # 8 per sample
# 'd' = DVE, 'p' = Pool(GPSIMD) for the per-head mask multiply.  Pool masks
# are slow (~5us) but run concurrently with the heads phase, so they go on
# EARLY heads; late heads (whose masks gate the @V phase) stay on the fast
# DVE.
MASK_ENG = "dddddddd"
# defer Q/K pairs 2-3 of proj(b) into attention(b)'s own filler list
DEFER_LATE = True

# ---------------------------------------------------------------- bass program
_CACHE = {}


def _build():
    if "nc" in _CACHE:
        return _CACHE["nc"]

    nc = bacc.Bacc(None, target_bir_lowering=False)
    xT_d = nc.declare_dram_parameter("xT", [BL, C, S], BF16, isOutput=False)
    wq_d = nc.declare_dram_parameter("wq_pad", [C, WQW], BF16, isOutput=False)
    wp_d = nc.declare_dram_parameter("wp_dense", [3, 128, C], BF16, isOutput=False)
    m_d = nc.declare_dram_parameter("m01", [128, BAND_W], BF16, isOutput=False)
    out_d = nc.declare_dram_parameter("out", [BL, S, C], BF16, isOutput=True)

    with tile.TileContext(nc) as tc:
        with (
            tc.tile_pool(name="singles", bufs=1) as singles,
            tc.tile_pool(name="xt_pool", bufs=3) as xt_pool,
            tc.tile_pool(name="out_pool", bufs=6) as out_pool,
            tc.tile_pool(name="den_pool", bufs=3) as den_pool,
            tc.tile_pool(name="ps_proj", bufs=4, space="PSUM") as ps_proj,
            tc.tile_pool(name="ps_big", bufs=2, space="PSUM") as ps_big,
        ):
            # ---- constants.  Q/K/V weight sections load as separate DMAs so
            # the first projection matmuls only wait for their own section.
            w_sb = singles.tile([128, 3, WQW], BF16)
            wq_v = wq_d.rearrange("(c p) w -> p c w", p=128)

            DNS = 2 * WPADQ  # dense qkv section base in w_sb

            def load_weights(part):
                # interleaved with the xt halves so the first projection
                # matmuls unblock as early as possible
                if part == 0:
                    nc.scalar.dma_start(w_sb[:, :, 0:128], wq_v[:, :, 0:128])
                    nc.scalar.dma_start(w_sb[:, :, 128:WPADQ], wq_v[:, :, 128:WPADQ])
                else:
                    nc.scalar.dma_start(
                        w_sb[:, :, WPADQ : 2 * WPADQ], wq_v[:, :, WPADQ : 2 * WPADQ]
                    )
                    # only the dense V section is read from the dense block
                    nc.scalar.dma_start(
                        w_sb[:, :, DNS + 2 * C :], wq_v[:, :, DNS + 2 * C :]
                    )

            wp_sb = singles.tile([128, 3, C], BF16)
            m_sb = singles.tile([128, BAND_W], BF16)

            def load_consts():
                # issued after proj(0) so these transfers cannot jump ahead
                # of the startup-critical xt/wQ DMAs on the DMA engines
                nc.sync.dma_start(m_sb, m_d[:, :])
                nc.sync.dma_start(wp_sb[:, :, :], wp_d.rearrange("f p c -> p f c"))

            _prj_n = [0]

            def prj_slot():
                _prj_n[0] += 1
                ps = ps_proj.tile([128, 512], F32, tag="mm", name=f"prj{_prj_n[0]}")
                return ps

            # ---- per-sample tiles, double-buffered for cross-sample overlap
            qTs, kTs, vvs, aoT3s, qksbs = [], [], [], [], []
            for i in range(2):
                qTs.append(singles.tile([128, 4, S], BF16, name=f"qT{i}"))
                kTs.append(singles.tile([128, 4, S], BF16, name=f"kT{i}"))
                qksbs.append(
                    singles.tile([128, 2, 4, NQT, 128], BF16, name=f"qksb{i}")
                )
                vvs.append(singles.tile([128, NQT, H, 49], BF16, name=f"vv{i}"))
                aoT3s.append(singles.tile([128, 3 * NQT, 128], BF16, name=f"aoT{i}"))
            expT = singles.tile([128, H, BAND_W], BF16, name="expT")
            ao_sb = singles.tile([128, NQT, H, D], BF16, name="ao")

            def setup_set(i):
                # ones column for the in-matmul softmax denominators
                nc.gpsimd.memset(vvs[i][:, :, :, D : D + 1], 1.0)

            def evac(engine, dst, src):
                if engine == "a":
                    nc.scalar.copy(dst, src)
                else:
                    nc.vector.tensor_copy(dst, src)

            def load_xt(b, split=False):
                xt = xt_pool.tile([128, 3, S], BF16)
                src = xT_d[b].rearrange("(c p) s -> p c s", p=128)
                if split:
                    # first ci chunk alone so the first projection matmul
                    # unblocks after a third of the transfer
                    nc.sync.dma_start(xt[:, 0, 0:512], src[:, 0, 0:512])
                    load_weights(0)
                    nc.sync.dma_start(xt[:, 1:3, 0:512], src[:, 1:3, 0:512])
                    nc.sync.dma_start(xt[:, :, 512:S], src[:, :, 512:S])
                    load_weights(1)
                else:
                    nc.sync.dma_start(xt[:, :, :], src)
                return xt

            def _qk_mm(xt, qk, pair, half, dstps):
                ncol = qk * WPADQ + pair * 128
                for ci in range(3):
                    nc.tensor.matmul(
                        dstps,
                        w_sb[:, ci, ncol : ncol + 128],
                        xt[:, ci, half * 512 : (half + 1) * 512],
                        start=(ci == 0),
                        stop=(ci == 2),
                    )

            def _v_mm(xt, st, dstps):
                for ci in range(3):
                    nc.tensor.matmul(
                        dstps,
                        xt[:, ci, st * 128 : (st + 1) * 128],
                        w_sb[:, ci, DNS + 2 * C : DNS + 3 * C],
                        start=(ci == 0),
                        stop=(ci == 2),
                    )

            def proj_units(b, xt):
                """QKV projection for sample b as narrow issue-units over the
                rotating psP slots.

                Sample 0 projects Q/K directly in the padded [dim, seq]
                orientation (3 x 512-col matmuls per 128-dim block).  Later
                samples project in the dense [seq, dim] orientation (25% fewer
                PE columns) into a staging buffer and rebuild the padded
                [dim, seq] qT/kT with one DMA xbar transpose per Q/K; the
                transposes overlap the previous sample's attention."""
                qT, kT, vv = qTs[b % 2], kTs[b % 2], vvs[b % 2]
                qksb = qksbs[b % 2]

                def qk_unit_pad(qk, pair, half, eng):
                    def u():
                        dst = qT if qk == 0 else kT
                        ps = prj_slot()
                        _qk_mm(xt, qk, pair, half, ps)
                        evac(
                            eng,
                            dst[:, pair, half * 512 : (half + 1) * 512],
                            ps,
                        )

                    u.half = half
                    return u

                def qk_unit_dns(qk, st, eng):
                    def u():
                        ps = prj_slot()
                        for ci in range(3):
                            nc.tensor.matmul(
                                ps[:, 0:C],
                                xt[:, ci, st * 128 : (st + 1) * 128],
                                w_sb[:, ci, DNS + qk * C : DNS + (qk + 1) * C],
                                start=(ci == 0),
                                stop=(ci == 2),
                            )
                        evac(
                            eng,
                            qksb[:, qk, :, st, :].rearrange(
                                "p j (s c) -> p j s c", s=2
                            )[:, :, :, 0:D],
                            ps[:, 0:C].rearrange("p (j s d) -> p j s d", j=4, s=2),
                        )

                    u.half = st // 4
                    return u

                def qk_transpose(qk):
                    def u():
                        dst = qT if qk == 0 else kT
                        nc.sync.dma_start_transpose(
                            dst[:, :, :].rearrange(
                                "p j (st c) -> p (j st) c", c=128
                            ),
                            qksb[:, qk, :, :, :].rearrange("p j st c -> p (j st c)"),
                        )

                    u.half = 3
                    return u

                def v_unit(st, eng):
                    def u():
                        ps = prj_slot()
                        _v_mm(xt, st, ps[:, 0:C])
                        evac(
                            eng,
                            vv[:, st, :, 0:D],
                            ps[:, 0:C].rearrange("p (h d) -> p h d", h=H),
                        )

                    u.half = 2 if st >= 4 else 1
                    return u

                ei = iter(QK_EVAC)
                early, late = [], []
                for qk in range(2):
                    for pair in range(4):
                        for half in range(2):
                            u = qk_unit_pad(qk, pair, half, next(ei))
                            (early if pair < 2 else late).append(u)
                vi = iter(V_EVAC)
                early.extend(v_unit(st, next(vi)) for st in range(NQT))
                return early, late

            def attention(b, head_fillers=(), must=(), tail=False):
                """Band QK^T + exp + mask per head, then per-q-tile @V with
                in-matmul denominators, batched normalize, and one DMA
                transpose to the dense aoT layout.  Filler units (next
                sample's projection / previous sample's output projection)
                drain after each head and between @V tiles so the PE always
                has independent work while Act/DVE chains complete.

                head_fillers drain through the heads phase (forced complete
                by its end -- they carry the next sample's Q/K projection
                whose DMA transposes must overlap the @V phase); av_fillers
                (V projection, prior out-proj) drain between @V tile-pairs."""
                fillers = list(must) + list(head_fillers)
                n_must = len(must)
                fi = [0]
                n_drain = 2 * H + NQT
                qT, kT, vv = qTs[b % 2], kTs[b % 2], vvs[b % 2]
                aoT3 = aoT3s[b % 2]

                def drain(k):
                    # heads-phase points get a 1.25x share: that is where the
                    # exp-paced PE stalls are; the @V phase is divide-paced
                    target = (k + 1) * 5 * len(fillers) // (4 * n_drain)
                    if k == 3 and n_must > target:
                        target = n_must
                    if k >= n_drain - 1:
                        target = len(fillers)
                    target = min(target, len(fillers))
                    while fi[0] < target:
                        fillers[fi[0]]()
                        fi[0] += 1

                drain_av = lambda k: drain(2 * H + k)

                for h in range(H):
                    pair, sub = divmod(h, 2)
                    p0 = sub * 64
                    # ---- QK^T exact band (5 one-bank slices) + exp
                    for si in range(N_SLICE):
                        sbase, sw = 1024 * si, SLICE_W[si]
                        pat = ps_big.tile([128, 1024], F32, tag="attn")
                        # the full extended band is computed: skipping the
                        # mask-zeroed extension columns would leave stale PSUM
                        # (possibly huge @V denominators) feeding exp -> inf,
                        # and inf * mask(0) = NaN.
                        for c, blo, bhi in SLICE_CHUNKS[si]:
                            # band col -> query: q = QLO[c] + (bandcol-OFFC[c])
                            a = blo
                            while a < bhi:
                                b2 = min(bhi, sbase + ((a - sbase) // 512 + 1) * 512)
                                nc.tensor.matmul(
                                    pat[:, a - sbase : b2 - sbase],
                                    kT[p0 : p0 + D, pair, c * 128 : (c + 1) * 128],
                                    qT[
                                        p0 : p0 + D,
                                        pair,
                                        QLO[c]
                                        + (a - OFFC[c]) : QLO[c]
                                        + (b2 - OFFC[c]),
                                    ],
                                    start=True,
                                    stop=True,
                                )
                                a = b2
                        nc.scalar.activation(
                            expT[:, h, sbase : sbase + sw],
                            pat[:, 0:sw],
                            mybir.ActivationFunctionType.Exp,
                            scale=SCALE,
                        )
                        if si == 1:
                            drain(2 * h)
                    # ---- window mask, one op per head (bf16 2x mode on DVE)
                    tt = (
                        nc.vector.tensor_tensor
                        if MASK_ENG[h] == "d"
                        else nc.gpsimd.tensor_tensor
                    )
                    tt(
                        expT[:, h, :],
                        expT[:, h, :],
                        m_sb[:, :],
                        mybir.AluOpType.mult,
                    )
                    drain(2 * h + 1)

                # ---- @V: two query tiles per [128,1024] PSUM tile (banks
                # 0/1), denominators in column 48 of each 49-wide head
                # section, batched normalize per tile-pair.
                n_tp = 2 if tail else 4  # transpose granularity (tiles/piece)
                for tp in range(NQT // 2):
                    po2t = ps_big.tile([128, 1024], F32, tag="attn")
                    po2v = po2t[:, :].rearrange("p (tt x) -> p tt x", tt=2)
                    for tt in range(2):
                        t = 2 * tp + tt
                        po2 = po2v[:, tt, 0 : H * 49].rearrange(
                            "p (h w) -> p h w", w=49
                        )
                        pieces = AV_PIECES[t]
                        for h in range(H):
                            for pi, (c, off, w) in enumerate(pieces):
                                nc.tensor.matmul(
                                    po2[0:w, h, :],
                                    expT[:, h, OFFC[c] + off : OFFC[c] + off + w],
                                    vv[:, c, h, :],
                                    start=(pi == 0),
                                    stop=(pi == len(pieces) - 1),
                                )
                    po2s = po2v[:, :, 0 : H * 49].rearrange(
                        "p tt (h w) -> p tt h w", w=49
                    )
                    den_r = den_pool.tile([128, 2, H], F32)
                    nc.vector.reciprocal(den_r[:, :, :], po2s[:, :, :, D])
                    nc.vector.tensor_tensor(
                        ao_sb[:, 2 * tp : 2 * tp + 2, :, :],
                        po2s[:, :, :, 0:D],
                        den_r[:, :, :].unsqueeze(3).broadcast_to([128, 2, H, D]),
                        mybir.AluOpType.mult,
                    )
                    if (2 * tp + 2) % n_tp == 0:
                        lo_t = (2 * tp + 2) - n_tp
                        nc.sync.dma_start_transpose(
                            aoT3[:, 3 * lo_t : 3 * (lo_t + n_tp), :],
                            ao_sb[:, lo_t : lo_t + n_tp, :, :].rearrange(
                                "p t h d -> p (t h d)"
                            ),
                        )
                    drain_av(2 * tp)
                    drain_av(2 * tp + 1)

            def _op_mm(aoT3, st, dstps):
                for j in range(3):
                    nc.tensor.matmul(
                        dstps,
                        aoT3[:, 3 * st + j, :],
                        wp_sb[:, j, :],
                        start=(j == 0),
                        stop=(j == 2),
                    )

            def out_proj_units(b):
                aoT3 = aoT3s[b % 2]
                units = []

                def st_unit(st, eng):
                    def u():
                        ot = out_pool.tile([128, C], BF16, tag="ot", name=f"ot{b}_{st}")
                        ps = prj_slot()
                        _op_mm(aoT3, st, ps[:, 0:C])
                        evac(eng, ot[:, :], ps[:, 0:C])
                        nc.sync.dma_start(
                            out_d[b, st * 128 : (st + 1) * 128, :], ot[:, :]
                        )

                    return u

                oi = iter(OUT_EVAC)
                for st in range(NQT):
                    units.append(st_unit(st, next(oi)))
                return units

            def out_proj_tail(b):
                # final sample: narrow evacs on alternating engines; the last
                # store pair splits across two DMA queues so the closing
                # transfers overlap
                aoT3 = aoT3s[b % 2]
                for sp in range(NQT // 2):
                    ot = out_pool.tile([128, 2, C], BF16)
                    for k, eng in ((0, "a"), (1, "d")):
                        st = 2 * sp + k
                        ps = prj_slot()
                        _op_mm(aoT3, st, ps[:, 0:C])
                        evac(eng, ot[:, k, :], ps[:, 0:C])
                        if sp == NQT // 2 - 1:
                            dma = nc.sync.dma_start if k == 0 else nc.scalar.dma_start
                            dma(out_d[b, st * 128 : (st + 1) * 128, :], ot[:, k, :])
                    if sp < NQT // 2 - 1:
                        nc.sync.dma_start(
                            out_d[b, 2 * sp * 128 : (2 * sp + 2) * 128, :].rearrange(
                                "(k p) c -> p k c", k=2
                            ),
                            ot[:, :, :],
                        )

            # ---------------- main pipeline
            xts = {0: load_xt(0, split=True)}
            setup_set(0)
            early0, late0 = proj_units(0, xts[0])
            # half-0 Q/K units first: they only need the first xt half
            early0.sort(key=lambda u: getattr(u, "half", 0))
            for u in early0:
                u()
            load_consts()
            setup_set(1)
            carry = late0
            for b in range(BL):
                must = list(carry)
                carry = []
                fillers = []
                if b + 1 < BL:
                    xts[b + 1] = load_xt(b + 1)  # prefetch during attention(b)
                    early, late = proj_units(b + 1, xts[b + 1])
                    fillers += early
                    carry = late
                if b > 0:
                    # interleave the previous sample's output-projection units
                    # among the projection units (round-robin)
                    ou = out_proj_units(b - 1)
                    mixed = []
                    k = max(1, len(fillers) // max(1, len(ou)))
                    oi2 = iter(ou)
                    for idx, u in enumerate(fillers):
                        mixed.append(u)
                        if idx % k == k - 1:
                            nu = next(oi2, None)
                            if nu is not None:
                                mixed.append(nu)
                    mixed.extend(oi2)
                    fillers = mixed
                attention(b, fillers, must=must, tail=(b == BL - 1))
            out_proj_tail(BL - 1)

    nc.finalize()
    _CACHE["nc"] = nc
    return nc


# ---------------------------------------------------------------- host wrapper
def _np_bf16(a):
    import ml_dtypes

    return np.asarray(a, dtype=ml_dtypes.bfloat16)


def _build_m01(mask):
    """[128, BAND_W] banded 0/1 mask in band layout (rows = key within
    chunk c, cols = q in [QLO[c], QHI[c]))."""
    mp = np.asarray(mask)[np.ix_(PERM, PERM)]
    good = np.isfinite(mp) & (mp == 0.0)
    m01 = np.zeros((128, BAND_W), np.float32)
    covered = 0
    for c in range(NQT):
        blk = good[QLO[c] : QHI[c], c * 128 : (c + 1) * 128]  # [q, k]
        m01[:, OFFC[c] : OFFC[c] + WC[c]] = blk.T.astype(np.float32)
        covered += int(blk.sum())
    assert covered == int(good.sum()), "mask not covered by band layout"
    return m01


def _pad_wqkv(w_qkv):
    """[384, 1152] -> [384, 2176]: padded Q/K (head h at 64h..64h+48) in
    cols [0:1024], then the dense natural w_qkv in cols [1024:2176]."""
    out = np.zeros((C, WQW), np.float32)
    for sec in range(2):  # Q, K
        for h in range(H):
            out[:, sec * WPADQ + h * 64 : sec * WPADQ + h * 64 + D] = w_qkv[
                :, sec * C + h * D : sec * C + (h + 1) * D
            ]
    out[:, 2 * WPADQ :] = w_qkv
    return out


def kernel(x, w_qkv, w_proj, b_proj, mask):
    x = np.asarray(x, np.float32)
    w_qkv = np.asarray(w_qkv, np.float32)
    w_proj = np.asarray(w_proj, np.float32)
    b_proj = np.asarray(b_proj, np.float32)

    nc = _build()

    xT = _np_bf16(np.ascontiguousarray(x[:, PERM, :].transpose(0, 2, 1)))  # [B, C, S']
    wq_pad = _np_bf16(_pad_wqkv(w_qkv))
    wp_dense = _np_bf16(np.ascontiguousarray(w_proj.reshape(3, 128, C)))
    m01 = _np_bf16(_build_m01(mask))

    in_maps = [
        {
            "xT": xT[c * BL : (c + 1) * BL],
            "wq_pad": wq_pad,
            "wp_dense": wp_dense,
            "m01": m01,
        }
        for c in range(N_CORES)
    ]
    res = run_bass_kernel_spmd(nc, in_maps, list(range(N_CORES)))
    out_p = np.concatenate(
        [np.asarray(res.results[c]["out"], np.float32) for c in range(N_CORES)], axis=0
    )
    out = np.empty_like(out_p)
    out[:, PERM, :] = out_p
    out += b_proj
    return out
